# revision 1
# baseline (speedup 1.0000x reference)
"""Capsule routing kernel — nn_Capsule_28097676051143 (Trainium2 / Bass / Tile).

kernel(u_vecs [64,512,256] f32, W [1,256,2048] f32) -> [64, 32, 64] f32.

Data-parallel: batch 64 split 8-per-core across 8 NeuronCores; W replicated.
The routing is algebraically refactored so the 268 MB
u_hat = einsum('bie,end->bnid') tensor is never materialized:

    a[n,e]  = sum_i c[n,i] u[i,e]            (c @ u)
    o[n,d]  = a[n,:] @ W[:, n-block]         (diagonal of small matmul)
    g[n,e]  = W[:, n-block] @ o_norm[n,:]    (block-diagonal matmul)
    b[i,n]  = u[i,:] @ g[n,:]                (u @ g^T; softmax over n)

exact up to fp reassociation, ~4x fewer FLOPs than materializing u_hat.
Heavy contractions run on the tensor engine in bf16 with fp32 PSUM
accumulation. Iteration 0 uses a stride-0 broadcast rhs for the uniform
softmax (b=0 -> c=1/N). All ACT work stays on one activation table
(exp/ln/copy): 1/sqrt(s2) is computed as exp(-0.5*ln(s2)).

On-chip layouts (per core, SBUF; p = 128 partitions):
    u_sb   [ip, (ic, b, e)]   lhsT for a-phase
    uT_sb  [ep, (ec, b, i)]   lhsT for b-phase
    w_sb   [ep, (ec, nd)]     lhsT for o-phase
    wT_sb  [ndp, (ndc, e)]    lhsT for g-phase
    o_flat [ndp, (ndc, b)]    o in flat capsule layout, nd = n*64+d
"""

import functools
import numpy as np
import ml_dtypes

B, I, E, N, D = 64, 512, 256, 32, 64
NCORES, BPC = 8, 8
ND = N * D  # 2048
BF = ml_dtypes.bfloat16


def _build_module():
    import concourse.bass as bass
    import concourse.bacc as bacc
    import concourse.mybir as mybir
    import concourse.tile as tile
    from contextlib import ExitStack

    F32 = mybir.dt.float32
    BF16 = mybir.dt.bfloat16
    AX = mybir.AxisListType
    AF = mybir.ActivationFunctionType

    nc = bacc.Bacc("TRN2", target_bir_lowering=False, debug=False,
                   num_devices=NCORES)

    u_in = nc.dram_tensor("u_in", [128, 4 * BPC * E], BF16, kind="ExternalInput")
    uT_in = nc.dram_tensor("uT_in", [128, 2 * BPC * I], BF16, kind="ExternalInput")
    w_in = nc.dram_tensor("w_in", [128, 2 * ND], BF16, kind="ExternalInput")
    wT_in = nc.dram_tensor("wT_in", [128, 16 * E], BF16, kind="ExternalInput")
    out_d = nc.dram_tensor("out_d", [BPC, N, D], F32, kind="ExternalOutput")

    ident_dram = nc.inline_tensor(np.eye(128, dtype=np.float32), name="ident_c")

    with tile.TileContext(nc) as tc, ExitStack() as ctx:
        cp = ctx.enter_context(tc.tile_pool(name="const", bufs=1))
        wk = ctx.enter_context(tc.tile_pool(name="work", bufs=2))
        # PSUM: big o-phase accumulators (2x2 banks) + small tiles (4x1 bank)
        pbig = ctx.enter_context(tc.tile_pool(name="pbig", bufs=2, space="PSUM"))
        pp = ctx.enter_context(tc.tile_pool(name="psum", bufs=4, space="PSUM"))

        # Load order matters: iteration 0 needs u (a-phase) then w (o-phase)
        # first; uT (b-phase) and wT (g-phase) are needed a few us later.
        # Split the big tensors so compute can start on the first chunks,
        # and spread across trigger engines to use multiple DMA queues.
        u_sb = cp.tile([128, 4 * BPC * E], BF16)
        for ic in range(4):
            nc.sync.dma_start(u_sb[:, ic * 2048:(ic + 1) * 2048],
                              u_in.ap()[:, ic * 2048:(ic + 1) * 2048])
        w_sb = cp.tile([128, 2 * ND], BF16)
        nc.scalar.dma_start(w_sb[:], w_in.ap())
        wT_sb = cp.tile([128, 16 * E], BF16)
        nc.scalar.dma_start(wT_sb[:], wT_in.ap())
        uT_sb = cp.tile([128, 2 * BPC * I], BF16)
        for ec in range(2):
            nc.scalar.dma_start(uT_sb[:, ec * 4096:(ec + 1) * 4096],
                                uT_in.ap()[:, ec * 4096:(ec + 1) * 4096])
        ident_sb = cp.tile([128, 128], F32)
        nc.sync.dma_start(ident_sb[:], ident_dram.ap())

        # ones2[:, 0] = 1 on p<64, ones2[:, 1] = 1 on p>=64 (s2 row sums)
        ones2 = cp.tile([128, 2], F32)
        nc.gpsimd.memset(ones2[:], 0.0)
        nc.gpsimd.memset(ones2[0:64, 0:1], 1.0)
        nc.gpsimd.memset(ones2[64:128, 1:2], 1.0)
        # uniform c for iteration 0
        c0 = cp.tile([128, 1], BF16)
        nc.gpsimd.memset(c0[:], 1.0 / N)

        def emit_a(get_c):
            """pat[p=e', (b, eh, n)] f32 = a^T; get_c(ic, b) -> [128, 32] rhs."""
            pat = pp.tile([128, 512], F32, tag="ps", name="pat")
            for b in range(8):
                for eh in range(2):
                    for ic in range(4):
                        nc.tensor.matmul(
                            pat[:, b * 64 + eh * N: b * 64 + (eh + 1) * N],
                            u_sb[:, ic * 2048 + b * E + eh * 128:
                                 ic * 2048 + b * E + (eh + 1) * 128],
                            get_c(ic, b),
                            start=(ic == 0), stop=(ic == 3),
                            skip_group_check=True)
            at = wk.tile([128, 2 * BPC * N], BF16, tag="at")
            nc.scalar.copy(
                at[:].rearrange("p (eh b n) -> p b eh n", eh=2, n=N),
                pat[:].rearrange("p (b eh n) -> p b eh n", eh=2, n=N))
            return at

        def emit_o(at):
            """o_flat[p, (ndc, b)] f32 <- diag blocks of a @ W."""
            o_flat = wk.tile([128, 16 * BPC], BF16, tag="oflat")
            for g in range(4):          # 4 ndc per big psum tile
                po = pbig.tile([128, 4 * BPC * N], F32, tag="po", name="po")
                for q in range(4):
                    ndc = g * 4 + q
                    for ec in range(2):
                        nc.tensor.matmul(
                            po[:, q * 256:(q + 1) * 256],
                            w_sb[:, ec * ND + ndc * 128: ec * ND + (ndc + 1) * 128],
                            at[:, ec * 256:(ec + 1) * 256],
                            start=(ec == 0), stop=(ec == 1),
                            skip_group_check=True)
                # diag extraction: src free = q*256 + b*32 + 2*(4g+q) + h
                #                        = q*258 + b*32 + (8g + h)
                base = po[:]
                pstep = base.ap[0][0]
                for h in range(2):
                    pv = slice(h * 64, (h + 1) * 64)
                    src = bass.AP(
                        tensor=base.tensor,
                        offset=base.offset + h * 64 * pstep + 8 * g + h,
                        ap=[[pstep, 64], [258, 4], [32, 8]])
                    dst = o_flat[pv, g * 32:(g + 1) * 32].rearrange(
                        "p (q b) -> p q b", b=8)
                    if h == 0:
                        nc.vector.tensor_copy(dst, src)
                    else:
                        nc.scalar.copy(dst, src)
            return o_flat

        def emit_s2(o_flat):
            """s2f psum [1, 256] (flat n*8+b): s2[n,b] = sum_d o[n,d,b]^2."""
            sq = wk.tile([128, 16 * BPC], F32, tag="sq")
            nc.vector.tensor_mul(sq[:], o_flat[:], o_flat[:])
            s2f = pp.tile([1, 256], F32, tag="ps", name="s2f")
            for ndc in range(16):
                for h in range(2):
                    nc.tensor.matmul(
                        s2f[:, (2 * ndc + h) * 8:(2 * ndc + h + 1) * 8],
                        ones2[:, h:h + 1],
                        sq[:, ndc * 8:(ndc + 1) * 8],
                        start=True, stop=True, skip_group_check=True)
            return s2f

        def emit_rs(o_flat):
            """rbe [128, (n*8+b)] f32 = 1/sqrt(s2) broadcast to all partitions."""
            s2f = emit_s2(o_flat)
            lnx = wk.tile([1, 256], F32, tag="lnx")
            nc.scalar.activation(lnx[:], s2f[:], AF.Ln)
            rsfl = wk.tile([1, 256], F32, tag="rsfl")   # exp(-0.5 ln) = rsqrt
            nc.scalar.activation(rsfl[:], lnx[:], AF.Exp, scale=-0.5)
            rbe = wk.tile([128, 256], F32, tag="rbe")
            nc.gpsimd.partition_broadcast(rbe[:], rsfl[:])
            return rbe

        def emit_g_raw(o_flat):
            """gt[p=e', (eh, b, n)] bf16 = W[:, n-blk] @ o_n (UNnormalized)."""
            # Z[p, (ndc, b, m')] bf16: block-diagonalized o
            Z = wk.tile([128, 16 * BPC * 2], BF16, tag="Z")
            nc.gpsimd.memset(Z[:], 0.0)
            zv = Z[:].rearrange("p (c two) -> p c two", two=2)
            nc.vector.tensor_copy(zv[0:64, :, 0], o_flat[0:64, :])
            nc.vector.tensor_copy(zv[64:128, :, 1], o_flat[64:128, :])
            pgt = pp.tile([128, 512], F32, tag="ps", name="pgt")
            for ndc in range(16):
                for eh in range(2):
                    nc.tensor.matmul(
                        pgt[:, eh * 256 + ndc * 16: eh * 256 + (ndc + 1) * 16],
                        wT_sb[:, ndc * 256 + eh * 128: ndc * 256 + (eh + 1) * 128],
                        Z[:, ndc * 16:(ndc + 1) * 16],
                        start=True, stop=True, skip_group_check=True)
            gt = wk.tile([128, 2 * BPC * N], BF16, tag="gt")
            for eh in range(2):
                dst = gt[:, eh * 256:(eh + 1) * 256].rearrange(
                    "p (b c two) -> p b c two", c=16, two=2)
                src = pgt[:, eh * 256:(eh + 1) * 256].rearrange(
                    "p (c b two) -> p b c two", b=8, two=2)
                nc.vector.tensor_copy(dst, src)
            return gt

        def emit_bnew_softmax(gt, rbe):
            """b = u @ g_raw^T, scaled by rs, softmax over n.

            Returns csb[p=i', (ic, b, n)] bf16."""
            pbt = [pp.tile([128, 512], F32, tag="ps", name=f"pbt{_j}")
                   for _j in range(2)]
            for j in range(2):
                for icr in range(2):
                    ic = 2 * j + icr
                    for b in range(8):
                        for eh in range(2):
                            nc.tensor.matmul(
                                pbt[j][:, icr * 256 + b * N: icr * 256 + (b + 1) * N],
                                uT_sb[:, eh * 4096 + b * I + ic * 128:
                                      eh * 4096 + b * I + (ic + 1) * 128],
                                gt[:, eh * 256 + b * N: eh * 256 + (b + 1) * N],
                                start=(eh == 0), stop=(eh == 1),
                                skip_group_check=True)
            esb = wk.tile([128, 4 * BPC * N], F32, tag="esb")
            ssum = wk.tile([128, 4 * BPC], F32, tag="ssum")
            rcp = wk.tile([128, 4 * BPC], F32, tag="rcp")
            csb = wk.tile([128, 4 * BPC * N], BF16, tag="csb")
            rin = rbe[:].rearrange("p (n b) -> p b n", b=8)
            for j in range(2):
                for icr in range(2):
                    pslice = pbt[j][:, icr * 256:(icr + 1) * 256]
                    nc.vector.tensor_mul(
                        pslice.rearrange("p (b n) -> p b n", n=N),
                        pslice.rearrange("p (b n) -> p b n", n=N), rin)
                nc.scalar.activation(esb[:, j * 512:(j + 1) * 512],
                                     pbt[j][:], AF.Exp)
                ev = esb[:, j * 512:(j + 1) * 512].rearrange(
                    "p (g n) -> p g n", n=N)
                nc.vector.reduce_sum(ssum[:, j * 16:(j + 1) * 16], ev, axis=AX.X)
                nc.vector.reciprocal(rcp[:, j * 16:(j + 1) * 16],
                                     ssum[:, j * 16:(j + 1) * 16])
                r3 = rcp[:, j * 16:(j + 1) * 16].rearrange(
                    "p (g o) -> p g o", o=1)
                e3b, r3b = bass.broadcast_tensor_aps(ev, r3)
                nc.vector.tensor_mul(
                    csb[:, j * 512:(j + 1) * 512].rearrange(
                        "p (g n) -> p g n", n=N), e3b, r3b)
            return csb

        def emit_final(o_flat):
            s2f = emit_s2(o_flat)
            lnx = wk.tile([1, 256], F32, tag="lnx")
            nc.scalar.activation(lnx[:], s2f[:], AF.Ln)
            r_s = wk.tile([1, 256], F32, tag="rsfl")    # sqrt(s2)
            nc.scalar.activation(r_s[:], lnx[:], AF.Exp, scale=0.5)
            onep = wk.tile([1, 256], F32, tag="onep")   # 1 + s2
            nc.scalar.add(onep[:], s2f[:], 1.0)
            rec = wk.tile([1, 256], F32, tag="rec")
            nc.vector.reciprocal(rec[:], onep[:])
            sclf = wk.tile([1, 256], F32, tag="sclf")   # sqrt(s2)/(1+s2)
            nc.vector.tensor_mul(sclf[:], r_s[:], rec[:])
            sbe = wk.tile([128, 256], F32, tag="rbe")
            nc.gpsimd.partition_broadcast(sbe[:], sclf[:])
            osc = wk.tile([128, 128], F32, tag="osc")
            for h in range(2):
                pv = slice(h * 64, (h + 1) * 64)
                sview = sbe[pv, :].rearrange("p (c g b) -> p c g b",
                                             g=2, b=8)[:, :, h, :]
                nc.vector.tensor_mul(
                    osc[pv, :].rearrange("p (c b) -> p c b", b=8),
                    o_flat[pv, :].rearrange("p (c b) -> p c b", b=8),
                    sview)
            ptr = pp.tile([128, 128], F32, tag="ps", name="ptr")
            nc.tensor.transpose(ptr[:], osc[:], ident_sb[:])
            trs = wk.tile([128, 128], F32, tag="trs")
            nc.vector.tensor_copy(trs[:], ptr[:])
            ov = out_d.ap().rearrange("b (c two) d -> two c b d", two=2)
            for h in range(2):
                nc.sync.dma_start(ov[h], trs[:, h * 64:(h + 1) * 64])

        def c_iter0(ic, b):
            donor = u_sb[:, 0:N].rearrange("p (o n) -> p o n", n=N)
            r = c0[:].rearrange("p (o n) -> p o n", n=1)
            _, rb = bass.broadcast_tensor_aps(donor, r)
            return rb

        get_c = c_iter0
        o_flat = None
        for it in range(3):
            at = emit_a(get_c)
            o_flat = emit_o(at)
            if it < 2:
                rbe = emit_rs(o_flat)
                gt = emit_g_raw(o_flat)
                csb = emit_bnew_softmax(gt, rbe)
                get_c = (lambda ic, b, csb=csb:
                         csb[:, ic * 256 + b * N: ic * 256 + (b + 1) * N])
        emit_final(o_flat)

    # The act-table chooser greedily resolves each function to the FIRST
    # set containing it, which flip-flops between exp_and_others and
    # natural_log_exp_and_others (Exp is in both; Ln only in the latter).
    # Present a view where Exp lives only in the shared exp+ln set so one
    # table load serves the whole kernel. Set ids keep their true indices.
    import concourse.hw_specs as hw_specs
    import concourse.bacc as bacc_mod
    orig_tables = hw_specs.get_activation_tables
    AF_ = mybir.ActivationFunctionType

    def patched_tables(arch):
        out = {}
        for name, s in orig_tables(arch).items():
            if name != "natural_log_exp_and_others":
                s = s - {AF_.Exp}
            out[name] = s
        return out

    hw_specs.get_activation_tables = patched_tables
    bacc_mod.get_activation_tables = patched_tables
    try:
        nc.compile()
    finally:
        hw_specs.get_activation_tables = orig_tables
        bacc_mod.get_activation_tables = orig_tables
    return nc


class _Runner:
    """Cached jitted SPMD executor (mirrors bass2jax.run_bass_via_pjrt)."""

    def __init__(self, nc):
        import jax
        import concourse.mybir as mybir
        from concourse import bass2jax
        from concourse.bass2jax import _bass_exec_p, install_neuronx_cc_hook
        from jax.sharding import Mesh, PartitionSpec
        from jax.experimental.shard_map import shard_map

        install_neuronx_cc_hook()
        self.jax = jax
        in_names, out_names, out_avals = [], [], []
        pname = nc.partition_id_tensor.name if nc.partition_id_tensor else None
        for alloc in nc.m.functions[0].allocations:
            if not isinstance(alloc, mybir.MemoryLocationSet):
                continue
            name = alloc.memorylocations[0].name
            if alloc.kind == "ExternalInput":
                if name != pname:
                    in_names.append(name)
            elif alloc.kind == "ExternalOutput":
                out_names.append(name)
                out_avals.append(jax.core.ShapedArray(
                    tuple(alloc.tensor_shape), mybir.dt.np(alloc.dtype)))
        self.in_names, self.out_names, self.out_avals = in_names, out_names, out_avals
        all_in = in_names + out_names + ([pname] if pname else [])
        n_params, n_outs = len(in_names), len(out_names)

        def _body(*args):
            operands = list(args)
            if pname is not None:
                operands.append(bass2jax.partition_id_tensor())
            return tuple(_bass_exec_p.bind(
                *operands, out_avals=tuple(out_avals), in_names=tuple(all_in),
                out_names=tuple(out_names), lowering_input_output_aliases=(),
                sim_require_finite=True, sim_require_nnan=True, nc=nc))

        devices = jax.devices()[:NCORES]
        mesh = Mesh(np.asarray(devices), ("core",))
        self._fn = jax.jit(
            shard_map(_body, mesh=mesh,
                      in_specs=(PartitionSpec("core"),) * (n_params + n_outs),
                      out_specs=(PartitionSpec("core"),) * n_outs,
                      check_rep=False),
            keep_unused=True)
        self._zeros = [np.zeros((NCORES * a.shape[0], *a.shape[1:]), a.dtype)
                       for a in out_avals]

    def run(self, per_core_inputs):
        concat = [np.concatenate([m[name] for m in per_core_inputs], axis=0)
                  for name in self.in_names]
        outs = self._fn(*concat, *self._zeros)
        self.jax.block_until_ready(outs)
        return [np.asarray(o) for o in outs]


@functools.lru_cache(maxsize=1)
def _get_runner():
    return _Runner(_build_module())


def _prep_inputs(u_vecs, W):
    u_vecs = np.ascontiguousarray(np.asarray(u_vecs, np.float32))
    W0 = np.ascontiguousarray(np.asarray(W, np.float32)[0])
    u4 = u_vecs.reshape(8, 8, 4, 128, 256).transpose(0, 3, 2, 1, 4) \
        .reshape(8, 128, 8192).astype(BF)
    uT4 = u_vecs.reshape(8, 8, 512, 2, 128).transpose(0, 4, 3, 1, 2) \
        .reshape(8, 128, 8192).astype(BF)
    w4 = W0.reshape(2, 128, 2048).transpose(1, 0, 2).reshape(128, 4096).astype(BF)
    wT4 = np.ascontiguousarray(W0.T).reshape(16, 128, 256) \
        .transpose(1, 0, 2).reshape(128, 4096).astype(BF)
    return [{"u_in": u4[c], "uT_in": uT4[c], "w_in": w4, "wT_in": wT4}
            for c in range(NCORES)]


def kernel(u_vecs: np.ndarray, W: np.ndarray) -> np.ndarray:
    runner = _get_runner()
    outs = runner.run(_prep_inputs(u_vecs, W))
    i = runner.out_names.index("out_d")
    return outs[i].reshape(B, N, D).astype(np.float32)

